# revision 15
# baseline (speedup 1.0000x reference)
"""DotGAT (2-layer dot-product graph attention) on 8 TRN2 NeuronCores.

Strategy (dst-sharded, degree-sorted, slot-major):
- Nodes globally sorted by in-degree, grouped into 49 "stripes" of 1024
  (128 dsts x 8 cores). Core c owns dsts ranked [k*1024+c*128, +128) of each
  stripe k. Edge "slot" (stripe k, partition p, slot s) holds the s-th
  in-edge of that dst, so the per-edge dst IS the partition index: segment
  softmax/aggregation become identity-lhsT PSUM accumulation + a free-axis
  reduce - no one-hot matmuls in the edge loop.
- Feature tables (FT = X@W per layer) are built in "AG order" (core-major
  rank order) in each core's HBM; src features are fetched with dma_gather.
  The int16 index limit is dodged by gathering PAIRS of adjacent table rows
  (idx = agrow//2 <= 25087); the parity select is folded into the exp-mask.
- Between layers an AllGather of H^T rebuilds the layer-2 table; X arrives
  host-transposed so PE never transposes activations.
"""
import json as _json
import os as _os
import numpy as np
_DEBUG1 = bool(int(_os.environ.get("K_DEBUG1", "0")))

# ---- shim: this walrus rejects >1 embedded sync-wait per instruction; hoist
# extras onto standalone EventSemaphore instructions (what wait_ge lowers to).
import concourse.bass as _cbass

if not getattr(_cbass.Bass, "_wait_split_patched", False):
    _orig_tjb = _cbass.Bass.to_json_bytes
    _ctr = [0]

    def _fix_block(insts):
        out = []
        for inst in insts:
            si = inst.get("sync_info")
            ow = (si or {}).get("on_wait") or []
            if si is not None and len(ow) > 1:
                for w in ow[:-1]:
                    _ctr[0] += 1
                    out.append({
                        "debug": inst.get("debug", 0),
                        "engine": inst["engine"],
                        "ins": [],
                        "name": f"WSPLIT-{_ctr[0]}-{inst['name']}",
                        "opcode": "EventSemaphore",
                        "outs": [],
                        "sync_info": {"on_update": [], "on_wait": [w]},
                    })
                si = dict(si)
                si["on_wait"] = [ow[-1]]
                inst = dict(inst)
                inst["sync_info"] = si
            out.append(inst)
        return out

    def _walk_fix(obj):
        if isinstance(obj, dict):
            if "instructions" in obj and isinstance(obj["instructions"], list):
                obj["instructions"] = _fix_block(obj["instructions"])
            for v in obj.values():
                _walk_fix(v)
        elif isinstance(obj, list):
            for v in obj:
                _walk_fix(v)

    def _patched_tjb(self, *a, **k):
        bir = _json.loads(_orig_tjb(self, *a, **k))
        _walk_fix(bir)
        return _json.dumps(bir).encode()

    _cbass.Bass.to_json_bytes = _patched_tjb
    _cbass.Bass._wait_split_patched = True

import concourse.bacc as bacc
import concourse.bass as bass
import concourse.mybir as mybir
from concourse.tile import TileContext
from concourse.tile_rust import add_dep_helper
from concourse._compat import get_trn_type
from concourse.bass_utils import run_bass_kernel_spmd
from concourse.library_config import mlp
from concourse.masks import make_identity

N, E, H, D = 50000, 500000, 4, 64
HD = H * D          # 256
P = 128
C = 8               # cores
STRIPE = P * C      # 1024
K_STRIPES = (N + STRIPE - 1) // STRIPE   # 49
NPAD = K_STRIPES * STRIPE                # 50176
OWN = K_STRIPES * P                      # 6272 rows per core
NIDX = 1024                              # slots per dma_gather call (8 tiles)
F32 = mybir.dt.float32
BF16 = mybir.dt.bfloat16
I16 = mybir.dt.int16


def _apx(base_ap, col0, dims):
    """AP at free-column col0 of a [128, F] tile with custom free dims."""
    return bass.AP(base_ap.tensor, base_ap.offset + col0,
                   [base_ap.ap[0]] + dims)


# ---------------------------------------------------------------- host prep
def _prepare(src, dst):
    deg = np.bincount(dst, minlength=N)
    perm = np.argsort(-deg, kind="stable").astype(np.int64)     # rank -> node
    ranks = np.arange(NPAD)
    k_of = ranks // STRIPE
    c_of = (ranks % STRIPE) // P
    p_of = ranks % P
    agrow_of_rank = c_of * OWN + k_of * P + p_of
    node_at_ag = np.zeros(NPAD, dtype=np.int64)
    node_at_ag[agrow_of_rank] = perm[np.minimum(ranks, N - 1)]
    ag_of_node = np.zeros(N, dtype=np.int64)
    ag_of_node[perm[np.arange(N)]] = agrow_of_rank[np.arange(N)]

    deg_r = np.zeros(NPAD, dtype=np.int64)
    deg_r[:N] = deg[perm]
    T = deg_r.reshape(K_STRIPES, STRIPE).max(axis=1)
    NT = int(T.sum())
    NCALLS = -(-NT // 8)
    NTP = NCALLS * 8
    Tp = T.copy()
    Tp[-1] += NTP - NT
    tile0 = np.zeros(K_STRIPES + 1, dtype=np.int64)
    np.cumsum(Tp, out=tile0[1:])

    order = np.argsort(dst, kind="stable")
    src_s = src[order]
    starts = np.zeros(N + 1, dtype=np.int64)
    np.cumsum(deg, out=starts[1:])

    gidx = np.zeros((C, P, NCALLS * 64), dtype=np.int16)
    mask8 = np.zeros((C, P, 8 * NTP), dtype=np.float32)
    for c in range(C):
        flat_idx = np.zeros(NTP * P, dtype=np.int16)   # slot i = t*128+p
        for k in range(K_STRIPES):
            Tk = int(Tp[k])
            base_t = int(tile0[k])
            r0 = k * STRIPE + c * P
            for p in range(P):
                r = r0 + p
                if r >= N:
                    continue
                node = perm[r]
                d0 = starts[node]
                g = min(int(deg[node]), Tk)
                ags = ag_of_node[src_s[d0:d0 + g]]
                colbase = 8 * base_t
                for t in range(g):
                    flat_idx[(base_t + t) * P + p] = ags[t] >> 1
                    b = int(ags[t] & 1)
                    for h in range(H):
                        mask8[c, p, colbase + h * 2 * Tk + 2 * t + b] = 1.0
        w = flat_idx.reshape(NCALLS, 64, 16)
        for call in range(NCALLS):
            gidx[c, :, call * 64:(call + 1) * 64] = np.tile(w[call].T, (8, 1))
    return dict(perm=perm, node_at_ag=node_at_ag, ag_of_node=ag_of_node,
                T=T, Tp=Tp, tile0=tile0, NT=NT, NCALLS=NCALLS, NTP=NTP,
                gidx=gidx, mask8=mask8)


# ------------------------------------------------------------- device build
def _build(meta):
    NCALLS, NTP = meta["NCALLS"], meta["NTP"]
    Tp, tile0 = meta["Tp"], meta["tile0"]
    EXCOLS = 8 * int(Tp.max())
    tile_stripe = np.zeros(NTP, dtype=np.int64)
    for k in range(K_STRIPES):
        tile_stripe[tile0[k]:tile0[k + 1]] = k

    nc = bacc.Bacc(get_trn_type() or "TRN2")
    xagT = nc.dram_tensor("xagT", [P, NPAD], F32, kind="ExternalInput")
    xownT = nc.dram_tensor("xownT", [P, OWN], F32, kind="ExternalInput")
    W1 = nc.dram_tensor("W1", [P, HD], F32, kind="ExternalInput")
    Wres1 = nc.dram_tensor("Wres1", [P, HD], F32, kind="ExternalInput")
    W2 = nc.dram_tensor("W2", [HD, HD], F32, kind="ExternalInput")
    gidx_d = nc.dram_tensor("gidx", [P, NCALLS * 64], I16, kind="ExternalInput")
    mask8_d = nc.dram_tensor("mask8", [P, 8 * NTP], F32, kind="ExternalInput")
    out_own = nc.dram_tensor("out_own", [OWN, HD], F32, kind="ExternalOutput")

    table1 = nc.dram_tensor("table1", [NPAD, HD], F32)
    table2 = nc.dram_tensor("table2", [NPAD, HD], F32)
    hownT = nc.dram_tensor("hownT", [2, P, OWN], F32)
    res_d = nc.dram_tensor("res_d", [OWN, HD], F32)
    hagT = nc.dram_tensor("hagT", [C, 2, P, OWN], F32, addr_space="Shared")

    MT = NPAD // P  # 392 m-tiles for table builds

    with TileContext(nc) as tc:
        with tc.tile_pool(name="const", bufs=1) as cpool, \
             tc.tile_pool(name="fd", bufs=1) as fdpool, \
             tc.tile_pool(name="tstage", bufs=6) as tstage, \
             tc.tile_pool(name="lhs", bufs=8) as lhspool, \
             tc.tile_pool(name="edge", bufs=3) as epool, \
             tc.tile_pool(name="small", bufs=8) as spool, \
             tc.tile_pool(name="exb", bufs=2) as expool, \
             tc.tile_pool(name="pstab", bufs=2, space="PSUM") as pstab, \
             tc.tile_pool(name="psagg", bufs=3, space="PSUM") as psagg, \
             tc.tile_pool(name="psfd", bufs=1, space="PSUM") as psfd, \
             tc.tile_pool(name="pstr", bufs=2, space="PSUM") as pstr:

            lib = nc.gpsimd.load_library(mlp)
            nidx_reg = nc.gpsimd.to_reg(NIDX)

            w1 = cpool.tile([P, HD], F32)
            nc.sync.dma_start(out=w1[:], in_=W1[:])
            wres = cpool.tile([P, HD], F32)
            nc.sync.dma_start(out=wres[:], in_=Wres1[:])
            w2a = cpool.tile([P, HD], F32)
            nc.sync.dma_start(out=w2a[:], in_=W2[0:P, :])
            w2b = cpool.tile([P, HD], F32)
            nc.sync.dma_start(out=w2b[:], in_=W2[P:HD, :])
            ident = cpool.tile([P, P], F32)
            make_identity(nc, ident[:])
            identb = cpool.tile([P, P], BF16)
            make_identity(nc, identb[:])
            gixt = cpool.tile([P, NCALLS * 64], I16)
            nc.sync.dma_start(out=gixt[:], in_=gidx_d[:])
            m8 = cpool.tile([P, 8 * NTP], F32)
            nc.sync.dma_start(out=m8[:], in_=mask8_d[:])
            xoT = cpool.tile([P, OWN], F32)
            nc.sync.dma_start(out=xoT[:], in_=xownT[:])

            fdbuf = fdpool.tile([P, K_STRIPES * HD], F32)

            def build_res():
                writes = []
                for k in range(K_STRIPES):
                    ps = psfd.tile([P, HD], F32, tag="fd")
                    nc.tensor.matmul(out=ps[:], lhsT=xoT[:, k * P:(k + 1) * P],
                                     rhs=wres[:], start=True, stop=True)
                    rst = tstage.tile([P, HD], F32, tag="tst")
                    nc.scalar.copy(out=rst[:], in_=ps[:])
                    wr = nc.scalar.dma_start(out=res_d[k * P:(k + 1) * P, :], in_=rst[:])
                    writes.append(wr.ins)
                return writes

            def build_table(layer, table, dep_insts=()):
                writes = []
                for j in range(MT):
                    ps = pstab.tile([P, HD], F32, tag="tab")
                    if layer == 1:
                        lt = lhspool.tile([P, P], F32, tag="lt")
                        nc.sync.dma_start(out=lt[:], in_=xagT[:, j * P:(j + 1) * P])
                        nc.tensor.matmul(out=ps[:], lhsT=lt[:], rhs=w1[:],
                                         start=True, stop=True)
                    else:
                        c = j // K_STRIPES
                        k = j % K_STRIPES
                        lt0 = lhspool.tile([P, P], F32, tag="lt")
                        d0 = nc.sync.dma_start(out=lt0[:],
                                               in_=hagT[c, 0, :, k * P:(k + 1) * P])
                        lt1 = lhspool.tile([P, P], F32, tag="lt")
                        d1 = nc.sync.dma_start(out=lt1[:],
                                               in_=hagT[c, 1, :, k * P:(k + 1) * P])
                        for dep in dep_insts:
                            add_dep_helper(d0.ins, dep, True, "cc->t2")
                            add_dep_helper(d1.ins, dep, True, "cc->t2")
                        nc.tensor.matmul(out=ps[:], lhsT=lt0[:], rhs=w2a[:],
                                         start=True, stop=False)
                        nc.tensor.matmul(out=ps[:], lhsT=lt1[:], rhs=w2b[:],
                                         start=False, stop=True)
                    st = tstage.tile([P, HD], F32, tag="tst")
                    nc.scalar.copy(out=st[:], in_=ps[:])
                    wr = nc.scalar.dma_start(out=table[j * P:(j + 1) * P, :], in_=st[:])
                    writes.append(wr.ins)
                return writes

            def build_fd(layer, dep_insts=()):
                for k in range(K_STRIPES):
                    ps = psfd.tile([P, HD], F32, tag="fd")
                    if layer == 1:
                        nc.tensor.matmul(out=ps[:], lhsT=xoT[:, k * P:(k + 1) * P],
                                         rhs=w1[:], start=True, stop=True)
                    else:
                        lt0 = lhspool.tile([P, P], F32, tag="lt")
                        d0 = nc.sync.dma_start(out=lt0[:],
                                               in_=hownT[0, :, k * P:(k + 1) * P])
                        lt1 = lhspool.tile([P, P], F32, tag="lt")
                        d1 = nc.sync.dma_start(out=lt1[:],
                                               in_=hownT[1, :, k * P:(k + 1) * P])
                        for dep in dep_insts:
                            add_dep_helper(d0.ins, dep, True, "hT->fd2")
                            add_dep_helper(d1.ins, dep, True, "hT->fd2")
                        nc.tensor.matmul(out=ps[:], lhsT=lt0[:], rhs=w2a[:],
                                         start=True, stop=False)
                        nc.tensor.matmul(out=ps[:], lhsT=lt1[:], rhs=w2b[:],
                                         start=False, stop=True)
                    nc.vector.tensor_copy(out=fdbuf[:, k * HD:(k + 1) * HD], in_=ps[:])

            def edge_phase(layer, table, barrier_insts, res_writes=()):
                tablev = table[:].rearrange("(a b) c -> a (b c)", b=2)
                cur = {}
                hT_writes = []

                def finalize(k):
                    Tk = int(Tp[k])
                    exm = cur["exm"]
                    agg = cur["agg"]
                    den = spool.tile([P, H], F32, tag="den")
                    nc.vector.tensor_reduce(
                        out=den[:],
                        in_=_apx(exm[:], 0, [[2 * Tk, H], [1, 2 * Tk]]),
                        axis=mybir.AxisListType.X, op=mybir.AluOpType.add)
                    rec = spool.tile([P, H], F32, tag="rec")
                    nc.vector.reciprocal(out=rec[:], in_=den[:])
                    st = tstage.tile([P, HD], F32, tag="fin")
                    nc.vector.tensor_tensor(
                        out=st[:].rearrange("p (h d) -> p h d", h=H),
                        in0=agg[:].rearrange("p (h d) -> p h d", h=H),
                        in1=_apx(rec[:], 0, [[1, H], [0, D]]),
                        op=mybir.AluOpType.mult)
                    if layer == 1:
                        rt = tstage.tile([P, HD], F32, tag="rln")
                        rld = nc.sync.dma_start(out=rt[:], in_=res_d[k * P:(k + 1) * P, :])
                        for bi in res_writes:
                            add_dep_helper(rld.ins, bi, True, "res->fin")
                        nc.vector.tensor_add(out=st[:], in0=st[:], in1=rt[:])
                        nc.vector.tensor_scalar_max(out=st[:], in0=st[:], scalar1=0.0)
                        for q in range(2):
                            tp = pstr.tile([P, P], F32, tag="tr")
                            nc.tensor.transpose(out=tp[:], in_=st[:, q * P:(q + 1) * P],
                                                identity=ident[:])
                            ts = tstage.tile([P, P], F32, tag="trs")
                            nc.scalar.copy(out=ts[:], in_=tp[:])
                            wr = nc.scalar.dma_start(
                                out=hownT[q, :, k * P:(k + 1) * P], in_=ts[:])
                            hT_writes.append(wr.ins)
                    else:
                        nc.sync.dma_start(out=out_own[k * P:(k + 1) * P, :], in_=st[:])

                for call in range(NCALLS):
                    fs2 = epool.tile([P, 8, 2 * HD], F32, tag="fs2", bufs=3)
                    g = nc.gpsimd.dma_gather(
                        fs2[:], tablev, gixt[:, call * 64:(call + 1) * 64],
                        NIDX, nidx_reg, 2 * HD, transpose=False,
                        single_packet=False)
                    add_dep_helper(g.ins, lib.ins, True, "lib->gather")
                    for bi in barrier_insts:
                        add_dep_helper(g.ins, bi, True, "table->gather")
                    for tl in range(8):
                        t = call * 8 + tl
                        k = int(tile_stripe[t])
                        t_local = t - int(tile0[k])
                        Tk = int(Tp[k])
                        if t_local == 0:
                            agg_t = psagg.tile([P, HD], F32, tag="agg")
                            cur["agg"] = agg_t
                            exm_t = expool.tile([P, EXCOLS], F32, tag="exm")
                            cur["exm"] = exm_t
                        agg = cur["agg"]
                        exm = cur["exm"]
                        fd_ap = _apx(fdbuf[:], k * HD, [[0, 2], [D, H], [1, D]])
                        prod = epool.tile([P, 2 * HD], F32, tag="prod")
                        nc.vector.tensor_tensor(
                            out=prod[:].rearrange("p (b h d) -> p b h d", b=2, h=H),
                            in0=fs2[:, tl, :].rearrange("p (b h d) -> p b h d",
                                                        b=2, h=H),
                            in1=fd_ap, op=mybir.AluOpType.mult)
                        sc = spool.tile([P, 2 * H], F32, tag="sc")
                        nc.vector.tensor_reduce(
                            out=sc[:].rearrange("p (b h) -> p b h", b=2),
                            in_=prod[:].rearrange("p (b h d) -> p b h d", b=2, h=H),
                            axis=mybir.AxisListType.X, op=mybir.AluOpType.add)
                        ex = spool.tile([P, 2 * H], F32, tag="ex")
                        nc.scalar.activation(out=ex[:], in_=sc[:],
                                             func=mybir.ActivationFunctionType.Exp,
                                             scale=0.125)
                        exm_ap = _apx(exm[:], 2 * t_local, [[1, 2], [2 * Tk, H]])
                        m8_ap = _apx(m8[:], 8 * int(tile0[k]) + 2 * t_local,
                                     [[1, 2], [2 * Tk, H]])
                        nc.vector.tensor_tensor(
                            out=exm_ap,
                            in0=ex[:].rearrange("p (b h) -> p b h", b=2),
                            in1=m8_ap, op=mybir.AluOpType.mult)
                        ws2 = epool.tile([P, 2 * HD], BF16, tag="ws2")
                        exw_ap = _apx(exm[:], 2 * t_local,
                                      [[1, 2], [2 * Tk, H], [0, D]])
                        nc.vector.tensor_tensor(
                            out=ws2[:].rearrange("p (b h d) -> p b h d", b=2, h=H),
                            in0=fs2[:, tl, :].rearrange("p (b h d) -> p b h d",
                                                        b=2, h=H),
                            in1=exw_ap, op=mybir.AluOpType.mult)
                        nc.tensor.matmul(out=agg[:], lhsT=identb[:], rhs=ws2[:, 0:HD],
                                         start=(t_local == 0), stop=False)
                        nc.tensor.matmul(out=agg[:], lhsT=identb[:], rhs=ws2[:, HD:],
                                         start=False, stop=(t_local == Tk - 1))
                        if t_local == Tk - 1:
                            finalize(k)
                return hT_writes

            # ---------------- layer 1 ----------------
            t1_writes = build_table(1, table1)
            build_fd(1)
            res_writes = build_res()
            hT_writes = edge_phase(1, table1, t1_writes, res_writes)

            # ---------------- allgather H^T ----------------
            if _DEBUG1:
                # debug: write layer-1 H (pre-transpose stages already in hownT);
                # dump hownT planes into out_own instead of running layer 2
                for k in range(K_STRIPES):
                    for q in range(2):
                        dt = tstage.tile([P, P], F32, tag="dbg")
                        dl = nc.sync.dma_start(out=dt[:], in_=hownT[q, :, k * P:(k + 1) * P])
                        for bi in hT_writes:
                            add_dep_helper(dl.ins, bi, True, "dbg")
                        tp2 = pstr.tile([P, P], F32, tag="tr")
                        nc.tensor.transpose(out=tp2[:], in_=dt[:], identity=ident[:])
                        ts2 = tstage.tile([P, P], F32, tag="dbg2")
                        nc.scalar.copy(out=ts2[:], in_=tp2[:])
                        nc.sync.dma_start(out=out_own[k * P:(k + 1) * P, q * P:(q + 1) * P], in_=ts2[:])
            if not _DEBUG1:
                cc = nc.gpsimd.collective_compute(
                    "AllGather", mybir.AluOpType.bypass,
                    replica_groups=[list(range(C))],
                    ins=[hownT[:]], outs=[hagT[:]])
                for wi in hT_writes:
                    add_dep_helper(cc.ins, wi, True, "hT->cc")

                # ---------------- layer 2 ----------------
                t2_writes = build_table(2, table2, dep_insts=(cc.ins,))
                build_fd(2, dep_insts=hT_writes)
                edge_phase(2, table2, t2_writes)

    nc.compile()
    return nc


_CACHE = {}


def _get_built(src, dst):
    key = (int(src[:16].sum()), int(dst[:16].sum()), int(src.sum()), int(dst.sum()))
    if key not in _CACHE:
        meta = _prepare(np.asarray(src, dtype=np.int64),
                        np.asarray(dst, dtype=np.int64))
        nc = _build(meta)
        _CACHE[key] = (meta, nc)
    return _CACHE[key]


def _run(x, src, dst, W1, Wres1, W2, trace=False):
    meta, nc = _get_built(np.asarray(src), np.asarray(dst))
    node_at_ag = meta["node_at_ag"]
    x = np.asarray(x, dtype=np.float32)
    x_ag = x[node_at_ag]                       # [NPAD, 128]
    xagT = np.ascontiguousarray(x_ag.T)        # [128, NPAD]
    in_maps = []
    for c in range(C):
        xownT = np.ascontiguousarray(x_ag[c * OWN:(c + 1) * OWN].T)
        in_maps.append({
            "xagT": xagT,
            "xownT": xownT,
            "W1": np.ascontiguousarray(np.asarray(W1, dtype=np.float32)),
            "Wres1": np.ascontiguousarray(np.asarray(Wres1, dtype=np.float32)),
            "W2": np.ascontiguousarray(np.asarray(W2, dtype=np.float32)),
            "gidx": meta["gidx"][c],
            "mask8": meta["mask8"][c],
        })
    res = run_bass_kernel_spmd(nc, in_maps, core_ids=list(range(C)), trace=trace)
    out = np.zeros((N, HD), dtype=np.float32)
    for c in range(C):
        rows = res.results[c]["out_own"]       # [OWN, 256], ag rows of core c
        nodes = node_at_ag[c * OWN:(c + 1) * OWN]
        loc = np.arange(OWN)
        rr = (loc // P) * STRIPE + c * P + (loc % P)   # global rank
        valid = rr < N
        out[nodes[valid]] = rows[valid]
    return out, res.exec_time_ns


def kernel(x, src, dst, W1, Wres1, W2):
    out, _ = _run(x, src, dst, W1, Wres1, W2, trace=False)
    return out


def kernel_traced(x, src, dst, W1, Wres1, W2):
    return _run(x, src, dst, W1, Wres1, W2, trace=True)


# revision 17
# speedup vs baseline: 1.0959x; 1.0959x over previous
"""DotGAT (2-layer dot-product graph attention) on 8 TRN2 NeuronCores.

Strategy (dst-sharded, degree-sorted, slot-major):
- Nodes globally sorted by in-degree, grouped into 49 "stripes" of 1024
  (128 dsts x 8 cores). Core c owns dsts ranked [k*1024+c*128, +128) of each
  stripe k. Edge "slot" (stripe k, partition p, slot s) holds the s-th
  in-edge of that dst, so the per-edge dst IS the partition index: segment
  softmax/aggregation become identity-lhsT PSUM accumulation + a free-axis
  reduce - no one-hot matmuls in the edge loop.
- Feature tables (FT = X@W per layer) are built in "AG order" (core-major
  rank order) in each core's HBM; src features are fetched with dma_gather.
  The int16 index limit is dodged by gathering PAIRS of adjacent table rows
  (idx = agrow//2 <= 25087); the parity select is folded into the exp-mask.
- Between layers an AllGather of H^T rebuilds the layer-2 table; X arrives
  host-transposed so PE never transposes activations.
"""
import json as _json
import os as _os
import numpy as np
_DEBUG1 = bool(int(_os.environ.get("K_DEBUG1", "0")))

# ---- shim: this walrus rejects >1 embedded sync-wait per instruction; hoist
# extras onto standalone EventSemaphore instructions (what wait_ge lowers to).
import concourse.bass as _cbass

if not getattr(_cbass.Bass, "_wait_split_patched", False):
    _orig_tjb = _cbass.Bass.to_json_bytes
    _ctr = [0]

    def _fix_block(insts):
        out = []
        for inst in insts:
            si = inst.get("sync_info")
            ow = (si or {}).get("on_wait") or []
            if si is not None and len(ow) > 1:
                for w in ow[:-1]:
                    _ctr[0] += 1
                    out.append({
                        "debug": inst.get("debug", 0),
                        "engine": inst["engine"],
                        "ins": [],
                        "name": f"WSPLIT-{_ctr[0]}-{inst['name']}",
                        "opcode": "EventSemaphore",
                        "outs": [],
                        "sync_info": {"on_update": [], "on_wait": [w]},
                    })
                si = dict(si)
                si["on_wait"] = [ow[-1]]
                inst = dict(inst)
                inst["sync_info"] = si
            out.append(inst)
        return out

    def _walk_fix(obj):
        if isinstance(obj, dict):
            if "instructions" in obj and isinstance(obj["instructions"], list):
                obj["instructions"] = _fix_block(obj["instructions"])
            for v in obj.values():
                _walk_fix(v)
        elif isinstance(obj, list):
            for v in obj:
                _walk_fix(v)

    def _patched_tjb(self, *a, **k):
        bir = _json.loads(_orig_tjb(self, *a, **k))
        _walk_fix(bir)
        return _json.dumps(bir).encode()

    _cbass.Bass.to_json_bytes = _patched_tjb
    _cbass.Bass._wait_split_patched = True

import concourse.bacc as bacc
import concourse.bass as bass
import concourse.mybir as mybir
from concourse.tile import TileContext
from concourse.tile_rust import add_dep_helper
from concourse._compat import get_trn_type
from concourse.bass_utils import run_bass_kernel_spmd
from concourse.library_config import mlp
from concourse.masks import make_identity

N, E, H, D = 50000, 500000, 4, 64
HD = H * D          # 256
P = 128
C = 8               # cores
STRIPE = P * C      # 1024
K_STRIPES = (N + STRIPE - 1) // STRIPE   # 49
NPAD = K_STRIPES * STRIPE                # 50176
OWN = K_STRIPES * P                      # 6272 rows per core
NIDX = 1024                              # slots per dma_gather call (8 tiles)
F32 = mybir.dt.float32
BF16 = mybir.dt.bfloat16
I16 = mybir.dt.int16


def _apx(base_ap, col0, dims):
    """AP at free-column col0 of a [128, F] tile with custom free dims."""
    return bass.AP(base_ap.tensor, base_ap.offset + col0,
                   [base_ap.ap[0]] + dims)


# ---------------------------------------------------------------- host prep
def _prepare(src, dst):
    deg = np.bincount(dst, minlength=N)
    perm = np.argsort(-deg, kind="stable").astype(np.int64)     # rank -> node
    ranks = np.arange(NPAD)
    k_of = ranks // STRIPE
    c_of = (ranks % STRIPE) // P
    p_of = ranks % P
    agrow_of_rank = c_of * OWN + k_of * P + p_of
    node_at_ag = np.zeros(NPAD, dtype=np.int64)
    node_at_ag[agrow_of_rank] = perm[np.minimum(ranks, N - 1)]
    ag_of_node = np.zeros(N, dtype=np.int64)
    ag_of_node[perm[np.arange(N)]] = agrow_of_rank[np.arange(N)]

    deg_r = np.zeros(NPAD, dtype=np.int64)
    deg_r[:N] = deg[perm]
    T = deg_r.reshape(K_STRIPES, STRIPE).max(axis=1)
    NT = int(T.sum())
    NCALLS = -(-NT // 8)
    NTP = NCALLS * 8
    Tp = T.copy()
    Tp[-1] += NTP - NT
    tile0 = np.zeros(K_STRIPES + 1, dtype=np.int64)
    np.cumsum(Tp, out=tile0[1:])

    order = np.argsort(dst, kind="stable")
    src_s = src[order]
    starts = np.zeros(N + 1, dtype=np.int64)
    np.cumsum(deg, out=starts[1:])

    gidx = np.zeros((C, P, NCALLS * 64), dtype=np.int16)
    mask8 = np.zeros((C, P, 8 * NTP), dtype=np.float32)
    for c in range(C):
        flat_idx = np.zeros(NTP * P, dtype=np.int16)   # slot i = t*128+p
        for k in range(K_STRIPES):
            Tk = int(Tp[k])
            base_t = int(tile0[k])
            r0 = k * STRIPE + c * P
            for p in range(P):
                r = r0 + p
                if r >= N:
                    continue
                node = perm[r]
                d0 = starts[node]
                g = min(int(deg[node]), Tk)
                ags = ag_of_node[src_s[d0:d0 + g]]
                colbase = 8 * base_t
                for t in range(g):
                    flat_idx[(base_t + t) * P + p] = ags[t] >> 1
                    b = int(ags[t] & 1)
                    for h in range(H):
                        mask8[c, p, colbase + h * 2 * Tk + 2 * t + b] = 1.0
        w = flat_idx.reshape(NCALLS, 64, 16)
        for call in range(NCALLS):
            gidx[c, :, call * 64:(call + 1) * 64] = np.tile(w[call].T, (8, 1))
    return dict(perm=perm, node_at_ag=node_at_ag, ag_of_node=ag_of_node,
                T=T, Tp=Tp, tile0=tile0, NT=NT, NCALLS=NCALLS, NTP=NTP,
                gidx=gidx, mask8=mask8)


# ------------------------------------------------------------- device build
def _build(meta):
    NCALLS, NTP = meta["NCALLS"], meta["NTP"]
    Tp, tile0 = meta["Tp"], meta["tile0"]
    EXCOLS = 8 * int(Tp.max())
    tile_stripe = np.zeros(NTP, dtype=np.int64)
    for k in range(K_STRIPES):
        tile_stripe[tile0[k]:tile0[k + 1]] = k

    nc = bacc.Bacc(get_trn_type() or "TRN2")
    xagT = nc.dram_tensor("xagT", [P, NPAD], F32, kind="ExternalInput")
    xownT = nc.dram_tensor("xownT", [P, OWN], F32, kind="ExternalInput")
    W1 = nc.dram_tensor("W1", [P, HD], F32, kind="ExternalInput")
    Wres1 = nc.dram_tensor("Wres1", [P, HD], F32, kind="ExternalInput")
    W2 = nc.dram_tensor("W2", [HD, HD], F32, kind="ExternalInput")
    gidx_d = nc.dram_tensor("gidx", [P, NCALLS * 64], I16, kind="ExternalInput")
    mask8_d = nc.dram_tensor("mask8", [P, 8 * NTP], F32, kind="ExternalInput")
    out_own = nc.dram_tensor("out_own", [OWN, HD], F32, kind="ExternalOutput")

    table1 = nc.dram_tensor("table1", [NPAD, HD], F32)
    table2 = nc.dram_tensor("table2", [NPAD, HD], F32)
    hownT = nc.dram_tensor("hownT", [2, P, OWN], F32)
    res_d = nc.dram_tensor("res_d", [OWN, HD], F32)
    hagT = nc.dram_tensor("hagT", [C, 2, P, OWN], F32, addr_space="Shared")

    MT = NPAD // P  # 392 m-tiles for table builds

    with TileContext(nc) as tc:
        with tc.tile_pool(name="const", bufs=1) as cpool, \
             tc.tile_pool(name="fd", bufs=1) as fdpool, \
             tc.tile_pool(name="tstage", bufs=6) as tstage, \
             tc.tile_pool(name="lhs", bufs=8) as lhspool, \
             tc.tile_pool(name="edge", bufs=3) as epool, \
             tc.tile_pool(name="small", bufs=8) as spool, \
             tc.tile_pool(name="exb", bufs=2) as expool, \
             tc.tile_pool(name="pstab", bufs=3, space="PSUM") as pstab, \
             tc.tile_pool(name="psagg", bufs=2, space="PSUM") as psagg, \
             tc.tile_pool(name="psfd", bufs=1, space="PSUM") as psfd, \
             tc.tile_pool(name="pstr", bufs=2, space="PSUM") as pstr:

            lib = nc.gpsimd.load_library(mlp)
            nidx_reg = nc.gpsimd.to_reg(NIDX)

            w1 = cpool.tile([P, HD], F32)
            nc.sync.dma_start(out=w1[:], in_=W1[:])
            wres = cpool.tile([P, HD], F32)
            nc.sync.dma_start(out=wres[:], in_=Wres1[:])
            w2a = cpool.tile([P, HD], F32)
            nc.sync.dma_start(out=w2a[:], in_=W2[0:P, :])
            w2b = cpool.tile([P, HD], F32)
            nc.sync.dma_start(out=w2b[:], in_=W2[P:HD, :])
            ident = cpool.tile([P, P], F32)
            make_identity(nc, ident[:])
            identb = cpool.tile([P, P], BF16)
            make_identity(nc, identb[:])
            gixt = cpool.tile([P, NCALLS * 64], I16)
            nc.sync.dma_start(out=gixt[:], in_=gidx_d[:])
            m8 = cpool.tile([P, 8 * NTP], F32)
            nc.sync.dma_start(out=m8[:], in_=mask8_d[:])
            xoT = cpool.tile([P, OWN], F32)
            nc.sync.dma_start(out=xoT[:], in_=xownT[:])

            fdbuf = fdpool.tile([P, K_STRIPES * HD], F32)

            def build_res():
                writes = []
                for k in range(K_STRIPES):
                    ps = psfd.tile([P, HD], F32, tag="fd")
                    nc.tensor.matmul(out=ps[:], lhsT=xoT[:, k * P:(k + 1) * P],
                                     rhs=wres[:], start=True, stop=True)
                    rst = tstage.tile([P, HD], F32, tag="tst")
                    nc.scalar.copy(out=rst[:], in_=ps[:])
                    wr = nc.scalar.dma_start(out=res_d[k * P:(k + 1) * P, :], in_=rst[:])
                    writes.append(wr.ins)
                return writes

            def build_table(layer, table, dep_insts=()):
                writes = []
                for j in range(MT):
                    ps = pstab.tile([P, HD], F32, tag="tab")
                    if layer == 1:
                        lt = lhspool.tile([P, P], F32, tag="lt")
                        nc.sync.dma_start(out=lt[:], in_=xagT[:, j * P:(j + 1) * P])
                        nc.tensor.matmul(out=ps[:], lhsT=lt[:], rhs=w1[:],
                                         start=True, stop=True)
                    else:
                        c = j // K_STRIPES
                        k = j % K_STRIPES
                        lt0 = lhspool.tile([P, P], F32, tag="lt")
                        d0 = nc.sync.dma_start(out=lt0[:],
                                               in_=hagT[c, 0, :, k * P:(k + 1) * P])
                        lt1 = lhspool.tile([P, P], F32, tag="lt")
                        d1 = nc.sync.dma_start(out=lt1[:],
                                               in_=hagT[c, 1, :, k * P:(k + 1) * P])
                        for dep in dep_insts:
                            add_dep_helper(d0.ins, dep, True, "cc->t2")
                            add_dep_helper(d1.ins, dep, True, "cc->t2")
                        nc.tensor.matmul(out=ps[:], lhsT=lt0[:], rhs=w2a[:],
                                         start=True, stop=False)
                        nc.tensor.matmul(out=ps[:], lhsT=lt1[:], rhs=w2b[:],
                                         start=False, stop=True)
                    st = tstage.tile([P, HD], F32, tag="tst")
                    nc.scalar.copy(out=st[:], in_=ps[:])
                    wr = nc.scalar.dma_start(out=table[j * P:(j + 1) * P, :], in_=st[:])
                    writes.append(wr.ins)
                return writes

            def build_fd(layer, dep_insts=()):
                for k in range(K_STRIPES):
                    ps = psfd.tile([P, HD], F32, tag="fd")
                    if layer == 1:
                        nc.tensor.matmul(out=ps[:], lhsT=xoT[:, k * P:(k + 1) * P],
                                         rhs=w1[:], start=True, stop=True)
                    else:
                        lt0 = lhspool.tile([P, P], F32, tag="lt")
                        d0 = nc.sync.dma_start(out=lt0[:],
                                               in_=hownT[0, :, k * P:(k + 1) * P])
                        lt1 = lhspool.tile([P, P], F32, tag="lt")
                        d1 = nc.sync.dma_start(out=lt1[:],
                                               in_=hownT[1, :, k * P:(k + 1) * P])
                        for dep in dep_insts:
                            add_dep_helper(d0.ins, dep, True, "hT->fd2")
                            add_dep_helper(d1.ins, dep, True, "hT->fd2")
                        nc.tensor.matmul(out=ps[:], lhsT=lt0[:], rhs=w2a[:],
                                         start=True, stop=False)
                        nc.tensor.matmul(out=ps[:], lhsT=lt1[:], rhs=w2b[:],
                                         start=False, stop=True)
                    nc.vector.tensor_copy(out=fdbuf[:, k * HD:(k + 1) * HD], in_=ps[:])

            def edge_phase(layer, table, barrier_insts, res_writes=()):
                tablev = table[:].rearrange("(a b) c -> a (b c)", b=2)
                cur = {}
                hT_writes = []

                def finalize(k):
                    Tk = int(Tp[k])
                    exm = cur["exm"]
                    agg = cur["agg"]
                    den = spool.tile([P, H], F32, tag="den")
                    nc.vector.tensor_reduce(
                        out=den[:],
                        in_=_apx(exm[:], 0, [[2 * Tk, H], [1, 2 * Tk]]),
                        axis=mybir.AxisListType.X, op=mybir.AluOpType.add)
                    rec = spool.tile([P, H], F32, tag="rec")
                    nc.vector.reciprocal(out=rec[:], in_=den[:])
                    st = tstage.tile([P, HD], F32, tag="fin")
                    nc.vector.tensor_tensor(
                        out=st[:].rearrange("p (h d) -> p h d", h=H),
                        in0=agg[:].rearrange("p (h d) -> p h d", h=H),
                        in1=_apx(rec[:], 0, [[1, H], [0, D]]),
                        op=mybir.AluOpType.mult)
                    if layer == 1:
                        rt = tstage.tile([P, HD], F32, tag="rln")
                        rld = nc.sync.dma_start(out=rt[:], in_=res_d[k * P:(k + 1) * P, :])
                        for bi in res_writes:
                            add_dep_helper(rld.ins, bi, True, "res->fin")
                        nc.vector.tensor_add(out=st[:], in0=st[:], in1=rt[:])
                        nc.vector.tensor_scalar_max(out=st[:], in0=st[:], scalar1=0.0)
                        for q in range(2):
                            tp = pstr.tile([P, P], F32, tag="tr")
                            nc.tensor.transpose(out=tp[:], in_=st[:, q * P:(q + 1) * P],
                                                identity=ident[:])
                            ts = tstage.tile([P, P], F32, tag="trs")
                            nc.scalar.copy(out=ts[:], in_=tp[:])
                            wr = nc.scalar.dma_start(
                                out=hownT[q, :, k * P:(k + 1) * P], in_=ts[:])
                            hT_writes.append(wr.ins)
                    else:
                        nc.sync.dma_start(out=out_own[k * P:(k + 1) * P, :], in_=st[:])

                for call in range(NCALLS):
                    fs2 = epool.tile([P, 8, 2 * HD], F32, tag="fs2", bufs=3)
                    g = nc.gpsimd.dma_gather(
                        fs2[:], tablev, gixt[:, call * 64:(call + 1) * 64],
                        NIDX, nidx_reg, 2 * HD, transpose=False,
                        single_packet=False)
                    add_dep_helper(g.ins, lib.ins, True, "lib->gather")
                    for bi in barrier_insts:
                        add_dep_helper(g.ins, bi, True, "table->gather")
                    for tl in range(8):
                        t = call * 8 + tl
                        k = int(tile_stripe[t])
                        t_local = t - int(tile0[k])
                        Tk = int(Tp[k])
                        if t_local == 0:
                            agg_t = psagg.tile([P, HD], F32, tag="agg")
                            cur["agg"] = agg_t
                            exm_t = expool.tile([P, EXCOLS], F32, tag="exm")
                            cur["exm"] = exm_t
                            fdd_t = expool.tile([P, 2 * HD], F32, tag="fdd")
                            nc.vector.tensor_copy(
                                out=fdd_t[:].rearrange("p (b c) -> p b c", b=2),
                                in_=_apx(fdbuf[:], k * HD, [[0, 2], [1, HD]]))
                            cur["fdd"] = fdd_t
                        agg = cur["agg"]
                        exm = cur["exm"]
                        prod = epool.tile([P, 2 * HD], F32, tag="prod")
                        nc.vector.tensor_tensor(
                            out=prod[:], in0=fs2[:, tl, :],
                            in1=cur["fdd"][:], op=mybir.AluOpType.mult)
                        sc = spool.tile([P, 2 * H], F32, tag="sc")
                        nc.vector.tensor_reduce(
                            out=sc[:].rearrange("p (b h) -> p b h", b=2),
                            in_=prod[:].rearrange("p (b h d) -> p b h d", b=2, h=H),
                            axis=mybir.AxisListType.X, op=mybir.AluOpType.add)
                        ex = spool.tile([P, 2 * H], F32, tag="ex")
                        nc.scalar.activation(out=ex[:], in_=sc[:],
                                             func=mybir.ActivationFunctionType.Exp,
                                             scale=0.125)
                        exm_ap = _apx(exm[:], 2 * t_local, [[1, 2], [2 * Tk, H]])
                        m8_ap = _apx(m8[:], 8 * int(tile0[k]) + 2 * t_local,
                                     [[1, 2], [2 * Tk, H]])
                        nc.vector.tensor_tensor(
                            out=exm_ap,
                            in0=ex[:].rearrange("p (b h) -> p b h", b=2),
                            in1=m8_ap, op=mybir.AluOpType.mult)
                        ws2 = epool.tile([P, 2 * HD], BF16, tag="ws2")
                        exw_ap = _apx(exm[:], 2 * t_local,
                                      [[1, 2], [2 * Tk, H], [0, D]])
                        nc.vector.tensor_tensor(
                            out=ws2[:].rearrange("p (b h d) -> p b h d", b=2, h=H),
                            in0=fs2[:, tl, :].rearrange("p (b h d) -> p b h d",
                                                        b=2, h=H),
                            in1=exw_ap, op=mybir.AluOpType.mult)
                        nc.tensor.matmul(out=agg[:], lhsT=identb[:], rhs=ws2[:, 0:HD],
                                         start=(t_local == 0), stop=False)
                        nc.tensor.matmul(out=agg[:], lhsT=identb[:], rhs=ws2[:, HD:],
                                         start=False, stop=(t_local == Tk - 1))
                        if t_local == Tk - 1:
                            finalize(k)
                return hT_writes

            # ---------------- layer 1 ----------------
            t1_writes = build_table(1, table1)
            build_fd(1)
            res_writes = build_res()
            hT_writes = edge_phase(1, table1, t1_writes, res_writes)

            # ---------------- allgather H^T ----------------
            if _DEBUG1:
                # debug: write layer-1 H (pre-transpose stages already in hownT);
                # dump hownT planes into out_own instead of running layer 2
                for k in range(K_STRIPES):
                    for q in range(2):
                        dt = tstage.tile([P, P], F32, tag="dbg")
                        dl = nc.sync.dma_start(out=dt[:], in_=hownT[q, :, k * P:(k + 1) * P])
                        for bi in hT_writes:
                            add_dep_helper(dl.ins, bi, True, "dbg")
                        tp2 = pstr.tile([P, P], F32, tag="tr")
                        nc.tensor.transpose(out=tp2[:], in_=dt[:], identity=ident[:])
                        ts2 = tstage.tile([P, P], F32, tag="dbg2")
                        nc.scalar.copy(out=ts2[:], in_=tp2[:])
                        nc.sync.dma_start(out=out_own[k * P:(k + 1) * P, q * P:(q + 1) * P], in_=ts2[:])
            if not _DEBUG1:
                cc = nc.gpsimd.collective_compute(
                    "AllGather", mybir.AluOpType.bypass,
                    replica_groups=[list(range(C))],
                    ins=[hownT[:]], outs=[hagT[:]])
                for wi in hT_writes:
                    add_dep_helper(cc.ins, wi, True, "hT->cc")

                # ---------------- layer 2 ----------------
                t2_writes = build_table(2, table2, dep_insts=(cc.ins,))
                build_fd(2, dep_insts=hT_writes)
                edge_phase(2, table2, t2_writes)

    nc.compile()
    return nc


_CACHE = {}


def _get_built(src, dst):
    key = (int(src[:16].sum()), int(dst[:16].sum()), int(src.sum()), int(dst.sum()))
    if key not in _CACHE:
        meta = _prepare(np.asarray(src, dtype=np.int64),
                        np.asarray(dst, dtype=np.int64))
        nc = _build(meta)
        _CACHE[key] = (meta, nc)
    return _CACHE[key]


def _run(x, src, dst, W1, Wres1, W2, trace=False):
    meta, nc = _get_built(np.asarray(src), np.asarray(dst))
    node_at_ag = meta["node_at_ag"]
    x = np.asarray(x, dtype=np.float32)
    x_ag = x[node_at_ag]                       # [NPAD, 128]
    xagT = np.ascontiguousarray(x_ag.T)        # [128, NPAD]
    in_maps = []
    for c in range(C):
        xownT = np.ascontiguousarray(x_ag[c * OWN:(c + 1) * OWN].T)
        in_maps.append({
            "xagT": xagT,
            "xownT": xownT,
            "W1": np.ascontiguousarray(np.asarray(W1, dtype=np.float32)),
            "Wres1": np.ascontiguousarray(np.asarray(Wres1, dtype=np.float32)),
            "W2": np.ascontiguousarray(np.asarray(W2, dtype=np.float32)),
            "gidx": meta["gidx"][c],
            "mask8": meta["mask8"][c],
        })
    res = run_bass_kernel_spmd(nc, in_maps, core_ids=list(range(C)), trace=trace)
    out = np.zeros((N, HD), dtype=np.float32)
    for c in range(C):
        rows = res.results[c]["out_own"]       # [OWN, 256], ag rows of core c
        nodes = node_at_ag[c * OWN:(c + 1) * OWN]
        loc = np.arange(OWN)
        rr = (loc // P) * STRIPE + c * P + (loc % P)   # global rank
        valid = rr < N
        out[nodes[valid]] = rows[valid]
    return out, res.exec_time_ns


def kernel(x, src, dst, W1, Wres1, W2):
    out, _ = _run(x, src, dst, W1, Wres1, W2, trace=False)
    return out


def kernel_traced(x, src, dst, W1, Wres1, W2):
    return _run(x, src, dst, W1, Wres1, W2, trace=True)
